# revision 49
# baseline (speedup 1.0000x reference)
"""Trainium2 Bass kernel for nn_AutoregressiveResidualBlock (dense_cnn).

Reference computation (per batch row, eval-mode BN, dilated queues of len 1):
    l1      = interleave(q1, x)                  # (bs, 1024), q1 = conv1_queue[0]
    h1      = relu(l1 @ w1.T + b1)
    h1bn    = h1 * s1 + t1                       # BN1 folded: s1 = g1/sqrt(v1+eps)
    l2      = interleave(q2, h1bn)               # (bs, 2048), q2 = conv2_queue[0]
    pre2    = l2 @ w2.T + b2 + l1 @ w_skip.T + bsk
    out     = relu(pre2) * s2 + t2               # BN2 folded

Device strategy (pure data-parallel over 8 cores, bs 16384 -> 2048/core):
  * Everything runs channel-major (channels on partitions).  The host
    pre-transposes activations into channel-major block tiles and
    transposes the output back: ZERO PE transposes on device.
  * interleave is eliminated by splitting every weight into even/odd column
    halves (even pairs with queue channels, odd with x / h1bn channels).
  * ALL matmuls are fp8(e4m3) DoubleRow at 0.5 cycles/row.  Precision comes
    from (hi,lo) error-feedback splits placed ONLY on the moving operand or
    as equal-magnitude stationary plane pairs (hardware loses ~half the lo
    refinement of mixed-magnitude stationary splits):
      - conv1:  moving = (hi,lo) split of x/q1 (host-packed), stationary =
                duplicated single-fp8 8*w1 planes.
      - conv2 h1 path: moving = (hi,lo) split of 8*relu(pre1+b1) computed
                on-chip (ACT evicts hi, DVE the f32 relu + the residual),
                stationary = duplicated fp8(8*s1*s2*w2-odd) planes.
      - conv2 q2 + skip paths: main pass = duplicated fp8(64*w) planes
                x (hi,lo)-split acts; plus a CORRECTION pass = paired
                lo-residual weight planes (equal magnitude -> exact on HW)
                x paired act-hi planes.  Net: ~bf16 weight precision and
                ~bf16 act precision at 1.5 DR-instructions per 128-chunk
                (vs 2.0 for bf16).
  * BN folds: s1 into w2-odd (host), t1 into the conv2 bias c2, s2 into all
    conv2/skip weights, t2 applied as a per-partition DVE add post-relu.
  * conv1(b+1) is emitted before conv2(b) so the h1 hi/lo evictions have a
    full block of slack before conv2 consumes them; the last block's conv2
    uses half/quarter-width psums so the tail eviction chain pipelines.
"""
import sys

sys.path.insert(0, "/opt/trn_rl_repo")

import numpy as np
import ml_dtypes
import concourse.bass as bass
import concourse.mybir as mybir
from concourse.tile import TileContext
from concourse.bass_utils import run_bass_kernel_spmd

P = 128
NCORES = 8
BS_FULL = 16384
BS = BS_FULL // NCORES   # 2048 rows per core
BLK = 512                # batch block (matmul moving free dim)
NB = BS // BLK           # 4
DIN = 512
MID = 1024
OUT = 512
KD = DIN // P            # 4  (x / q1 channel chunks)
KM = MID // P            # 8  (q2 / h1 channel chunks)
MT = MID // P            # 8  conv1 out tiles
OT = OUT // P            # 4  conv2 out tiles
EPS = 1e-5

SW1 = 8.0    # conv1 fp8 weight range scale (psum1 = SW1*pre1)
SH = 8.0     # h1 tile scale (tiles hold SH*relu(pre1+b1))
SW2 = 64.0   # conv2 psum scale (psum2 = SW2*s2*pre2)

f32 = mybir.dt.float32
f8 = mybir.dt.float8e4
bf16 = mybir.dt.bfloat16
NP8 = ml_dtypes.float8_e4m3
RELU = mybir.ActivationFunctionType.Relu
ADD = mybir.AluOpType.add
MAX = mybir.AluOpType.max
SUB = mybir.AluOpType.subtract
DR = mybir.MatmulPerfMode.DoubleRow

_nc_cache = [None]


# --------------------------------------------------------------------------
# wait-splitting post-pass: this container's walrus rejects >1 inline sem wait
# on several opcodes (Matmult: 1; CTRL NoOp/Drain: ~4).  Hoist excess waits
# onto same-engine NoOps inserted immediately before the instruction —
# semantically identical (the engine blocks at the NoOp instead).
_wfix_counter = [0]


def _fix_block_waits(b, cap, nop_cap):
    il = b.instructions
    i = 0
    while i < len(il):
        inst = il[i]
        body = getattr(inst, 'body_bb', None)
        if body is not None:
            _fix_block_waits(body, cap, nop_cap)
        si = inst.sync_info
        if si is None:
            i += 1
            continue
        w = list(si.on_wait or [])
        if len(w) <= cap:
            i += 1
            continue
        keep = w[-cap:]
        excess = w[:-cap]
        nops = []
        for j in range(0, len(excess), nop_cap):
            chunk = excess[j:j + nop_cap]
            _wfix_counter[0] += 1
            nop = mybir.InstNoOp(name=f"I-wfix-{_wfix_counter[0]}", ins=[], outs=[])
            nop.engine = inst.engine
            nop.sync_info = mybir.SyncInfo(on_wait=chunk, on_update=[])
            nops.append(nop)
        si.on_wait = keep
        inst.sync_info = si
        il[i:i] = nops
        i += len(nops) + 1


def fix_waits(nc, cap=1, nop_cap=1):
    for b in nc.m.functions[0].blocks:
        _fix_block_waits(b, cap, nop_cap)
    return nc


# --------------------------------------------------------------------------
def build_nc():
    nc = bass.Bass()
    # wide paired act tiles: [NB, npairs, P, 2(chunk-in-pair), 2(hi/lo), BLK]
    # slice [:, j, :, :] = chunk 2t+j's (hi,lo) planes (conv1/skip/q2 main
    # passes); slice [:, :, 0, :] = the pair's (hi_c, hi_c1) planes
    # (correction passes).
    xw_d = nc.declare_dram_parameter("xw", [NB, KD // 2, P, 2, 2, BLK], f8,
                                     isOutput=False)
    q1w_d = nc.declare_dram_parameter("q1w", [NB, KD // 2, P, 2, 2, BLK], f8,
                                      isOutput=False)
    q2w_d = nc.declare_dram_parameter("q2w", [NB, KM // 2, P, 2, 2, BLK], f8,
                                      isOutput=False)
    # conv1 / h-path weights: duplicated single-fp8 planes
    w1q_d = nc.declare_dram_parameter("w1q8", [KD, P, 2, MID], f8, isOutput=False)
    w1x_d = nc.declare_dram_parameter("w1x8", [KD, P, 2, MID], f8, isOutput=False)
    w2h_d = nc.declare_dram_parameter("w2h8", [KM, P, 2, OUT], f8, isOutput=False)
    # q2 / skip main weights (dup planes) + paired lo-residual corrections
    w2qd_d = nc.declare_dram_parameter("w2qd", [KM, P, 2, OUT], f8, isOutput=False)
    w2ql_d = nc.declare_dram_parameter("w2ql", [KM // 2, P, 2, OUT], f8,
                                       isOutput=False)
    wsqd_d = nc.declare_dram_parameter("wsqd", [KD, P, 2, OUT], f8, isOutput=False)
    wsql_d = nc.declare_dram_parameter("wsql", [KD // 2, P, 2, OUT], f8,
                                       isOutput=False)
    wsxd_d = nc.declare_dram_parameter("wsxd", [KD, P, 2, OUT], f8, isOutput=False)
    wsxl_d = nc.declare_dram_parameter("wsxl", [KD // 2, P, 2, OUT], f8,
                                       isOutput=False)
    b1v_d = nc.declare_dram_parameter("b1v", [P, MT], f32, isOutput=False)
    s2c2v_d = nc.declare_dram_parameter("s2c2v", [P, OT], f32, isOutput=False)
    t2v_d = nc.declare_dram_parameter("t2v", [P, OT], f32, isOutput=False)
    out_d = nc.declare_dram_parameter("outT", [OUT, BS], f32, isOutput=True)

    with TileContext(nc) as tc:
        with (
            tc.tile_pool(name="wpool", bufs=1) as wpool,
            tc.tile_pool(name="const", bufs=1) as const,
            tc.tile_pool(name="actA", bufs=2) as actA,     # x/q1 wide tiles
            tc.tile_pool(name="actB", bufs=2) as actB,     # q2 wide tiles
            tc.tile_pool(name="hpool", bufs=2) as hpool,   # h1 hi-lo tiles
            tc.tile_pool(name="tpool", bufs=2) as tpool,   # f32 relu tmp
            tc.tile_pool(name="zpool", bufs=2) as zpool,   # conv2 evictions
            tc.tile_pool(name="opool", bufs=2) as opool,
            tc.tile_pool(name="psum1", bufs=4, space="PSUM") as psum1,
            tc.tile_pool(name="psum2", bufs=3, space="PSUM") as psum2,
        ):
            def load_acts(b):
                q1w, xw, q2w = [], [], []
                for t in range(KD // 2):
                    tl = actA.tile([P, 2, 2, BLK], f8, tag=f"q1w{t}",
                                   name=f"q1w{t}_{b}")
                    nc.sync.dma_start(out=tl[:], in_=q1w_d[b, t])
                    q1w.append(tl)
                for t in range(KD // 2):
                    tl = actA.tile([P, 2, 2, BLK], f8, tag=f"xw{t}",
                                   name=f"xw{t}_{b}")
                    nc.sync.dma_start(out=tl[:], in_=xw_d[b, t])
                    xw.append(tl)
                for t in range(KM // 2):
                    tl = actB.tile([P, 2, 2, BLK], f8, tag=f"q2w{t}",
                                   name=f"q2w{t}_{b}")
                    nc.sync.dma_start(out=tl[:], in_=q2w_d[b, t])
                    q2w.append(tl)
                return q1w, xw, q2w

            # PE warm-up: chain of dummy matmuls bridges the initial DMA wait
            wup = const.tile([P, P], bf16)
            nc.vector.memset(wup[:], 0)
            psw = psum2.tile([P, BLK], f32, tag="p2", name="psw")
            for i in range(28):
                nc.tensor.matmul(psw[:, 0:P], wup[:], wup[:],
                                 start=True, stop=True)

            # ---- resident weights (conv1 first, split across queues) ----
            def wload(dram, n, shape, tag, engine):
                ts = []
                for c in range(n):
                    t = wpool.tile(shape, f8, tag=f"{tag}{c}")
                    engine.dma_start(out=t[:], in_=dram[c])
                    ts.append(t)
                return ts

            w1q = wload(w1q_d, KD, [P, 2, MID], "w1q", nc.gpsimd)
            w1x = wload(w1x_d, KD, [P, 2, MID], "w1x", nc.scalar)

            # ---- constants (before the bulky conv2 weights) ----
            b1v = const.tile([P, MT], f32)
            nc.scalar.dma_start(out=b1v[:], in_=b1v_d[:])
            s2c2v = const.tile([P, OT], f32)
            nc.scalar.dma_start(out=s2c2v[:], in_=s2c2v_d[:])
            t2v = const.tile([P, OT], f32)
            nc.scalar.dma_start(out=t2v[:], in_=t2v_d[:])

            acts = {}
            acts[0] = load_acts(0)

            w2qd = wload(w2qd_d, KM, [P, 2, OUT], "w2qd", nc.gpsimd)
            w2ql = wload(w2ql_d, KM // 2, [P, 2, OUT], "w2ql", nc.gpsimd)
            wsqd = wload(wsqd_d, KD, [P, 2, OUT], "wsqd", nc.scalar)
            wsql = wload(wsql_d, KD // 2, [P, 2, OUT], "wsql", nc.scalar)
            wsxd = wload(wsxd_d, KD, [P, 2, OUT], "wsxd", nc.scalar)
            wsxl = wload(wsxl_d, KD // 2, [P, 2, OUT], "wsxl", nc.scalar)
            w2h = wload(w2h_d, KM, [P, 2, OUT], "w2h", nc.gpsimd)

            h1_tiles = {}

            def conv1(b):
                q1w, xw, q2w = acts[b]
                hts = [hpool.tile([P, 2, BLK], f8, tag=f"h1{m}",
                                  name=f"h1{m}_{b}") for m in range(MT)]
                for m in range(MT):
                    ps = psum1.tile([P, BLK], f32, tag="p1", name=f"p1_{b}_{m}")
                    for c in range(KD):
                        nc.tensor.matmul(ps[:], w1q[c][:, :, m * P:(m + 1) * P],
                                         q1w[c // 2][:, c % 2, :, :],
                                         start=(c == 0), stop=False,
                                         perf_mode=DR)
                    for c in range(KD):
                        nc.tensor.matmul(ps[:], w1x[c][:, :, m * P:(m + 1) * P],
                                         xw[c // 2][:, c % 2, :, :],
                                         start=False, stop=(c == KD - 1),
                                         perf_mode=DR)
                    # h tiles hold SH*relu(pre1+b1) as (hi, lo) fp8 planes;
                    # psum = SW1*pre1 and SW1 == SH.
                    hi = hts[m][:, 0, :]
                    nc.scalar.activation(hi, ps[:], RELU, bias=b1v[:, m:m + 1])
                    tmp = tpool.tile([P, BLK], f32, tag=f"tmp{m % 2}",
                                     name=f"tmp{b}_{m}")
                    nc.vector.tensor_scalar(out=tmp[:], in0=ps[:],
                                            scalar1=b1v[:, m:m + 1],
                                            scalar2=0.0, op0=ADD, op1=MAX)
                    nc.vector.tensor_tensor(out=hts[m][:, 1, :], in0=tmp[:],
                                            in1=hi, op=SUB)
                h1_tiles[b] = hts

            def conv2(b):
                q1w, xw, q2w = acts[b]
                hts = h1_tiles[b]
                for o in range(OT):
                    osl = slice(o * P, (o + 1) * P)
                    if b < NB - 1:
                        W = BLK
                    elif o < OT - 1:
                        W = BLK // 2
                    else:
                        W = BLK // 4   # last tile: shortest eviction tail
                    for hp in range(BLK // W):
                        cs = slice(hp * W, (hp + 1) * W)
                        ps = psum2.tile([P, W], f32, tag="p2",
                                        name=f"p2_{b}_{o}_{hp}")
                        # q2 main (dup W x hi/lo acts) + lo-pair correction
                        for c in range(KM):
                            nc.tensor.matmul(ps[:], w2qd[c][:, :, osl],
                                             q2w[c // 2][:, c % 2, :, cs],
                                             start=(c == 0), stop=False,
                                             perf_mode=DR)
                        for t in range(KM // 2):
                            nc.tensor.matmul(ps[:], w2ql[t][:, :, osl],
                                             q2w[t][:, :, 0, cs],
                                             start=False, stop=False,
                                             perf_mode=DR)
                        # skip main + corrections
                        for c in range(KD):
                            nc.tensor.matmul(ps[:], wsqd[c][:, :, osl],
                                             q1w[c // 2][:, c % 2, :, cs],
                                             start=False, stop=False,
                                             perf_mode=DR)
                        for t in range(KD // 2):
                            nc.tensor.matmul(ps[:], wsql[t][:, :, osl],
                                             q1w[t][:, :, 0, cs],
                                             start=False, stop=False,
                                             perf_mode=DR)
                        for c in range(KD):
                            nc.tensor.matmul(ps[:], wsxd[c][:, :, osl],
                                             xw[c // 2][:, c % 2, :, cs],
                                             start=False, stop=False,
                                             perf_mode=DR)
                        for t in range(KD // 2):
                            nc.tensor.matmul(ps[:], wsxl[t][:, :, osl],
                                             xw[t][:, :, 0, cs],
                                             start=False, stop=False,
                                             perf_mode=DR)
                        # h1 last: conv1 evictions get maximal slack
                        for c in range(KM):
                            nc.tensor.matmul(ps[:], w2h[c][:, :, osl],
                                             hts[c][:, :, cs],
                                             start=False, stop=(c == KM - 1),
                                             perf_mode=DR)
                        zb = zpool.tile([P, W], f32,
                                        tag=f"zb{o % 2}" if W == BLK
                                        else f"zbh{hp}",
                                        name=f"zb{b}_{o}_{hp}")
                        nc.scalar.activation(zb[:], ps[:], RELU,
                                             scale=1.0 / SW2,
                                             bias=s2c2v[:, o:o + 1])
                        ob = opool.tile([P, W], f32,
                                        tag=f"ob{o % 2}" if W == BLK
                                        else f"obh{hp}",
                                        name=f"ob{b}_{o}_{hp}")
                        nc.vector.tensor_scalar(out=ob[:], in0=zb[:],
                                                scalar1=t2v[:, o:o + 1],
                                                scalar2=None, op0=ADD)
                        seng = nc.gpsimd if (W == BLK or (o * 2 + hp) % 2 == 0) \
                            else nc.sync
                        seng.dma_start(
                            out=out_d[osl,
                                      b * BLK + hp * W: b * BLK + (hp + 1) * W],
                            in_=ob[:])

            # software-pipelined emission: conv2(b) after conv1(b+1) so conv1
            # evictions have a full conv1-block of slack before conv2 uses them
            conv1(0)
            for b in range(1, NB):
                acts[b] = load_acts(b)
                conv1(b)
                conv2(b - 1)
            conv2(NB - 1)
    fix_waits(nc)
    return nc


def _get_nc():
    if _nc_cache[0] is None:
        _nc_cache[0] = build_nc()
    return _nc_cache[0]


# --------------------------------------------------------------------------
def _split8(a):
    hi = a.astype(NP8)
    lo = (a - hi.astype(np.float32)).astype(NP8)
    return hi, lo


def _host_prep(inputs):
    x = np.ascontiguousarray(inputs["x"][:, :, 0], dtype=np.float32)
    q1 = np.ascontiguousarray(inputs["conv1_queue"][0, :, :, 0], dtype=np.float32)
    q2 = np.ascontiguousarray(inputs["conv2_queue"][0, :, :, 0], dtype=np.float32)
    w1 = np.asarray(inputs["w1"], dtype=np.float32)
    w2 = np.asarray(inputs["w2"], dtype=np.float32)
    ws = np.asarray(inputs["w_skip"], dtype=np.float32)
    b1 = np.asarray(inputs["b1"], dtype=np.float32)
    b2 = np.asarray(inputs["b2"], dtype=np.float32)
    bsk = np.asarray(inputs["b_skip"], dtype=np.float32)

    s1 = (inputs["bn1_scale"] / np.sqrt(inputs["bn1_var"] + EPS)).astype(np.float32)
    t1 = (inputs["bn1_bias"] - inputs["bn1_mean"] * s1).astype(np.float32)
    s2 = (inputs["bn2_scale"] / np.sqrt(inputs["bn2_var"] + EPS)).astype(np.float32)
    t2 = (inputs["bn2_bias"] - inputs["bn2_mean"] * s2).astype(np.float32)
    c2 = (b2 + w2[:, 1::2] @ t1 + bsk).astype(np.float32)

    # K-major scaled weight matrices (in_ch, out_cols)
    w1q_m = np.ascontiguousarray(w1[:, 0::2].T) * SW1
    w1x_m = np.ascontiguousarray(w1[:, 1::2].T) * SW1
    w2q_m = np.ascontiguousarray((w2[:, 0::2] * s2[:, None]).T) * SW2
    w2h_m = np.ascontiguousarray(
        (w2[:, 1::2] * (s2[:, None] * s1[None, :])).T) * (SW2 / SH)
    wsq_m = np.ascontiguousarray((ws[:, 0::2] * s2[:, None]).T) * SW2
    wsx_m = np.ascontiguousarray((ws[:, 1::2] * s2[:, None]).T) * SW2

    def wdup8(wm):    # duplicated single-fp8 planes: [C//P, P, 2, ncols]
        q = wm.astype(NP8).reshape(wm.shape[0] // P, P, wm.shape[1])
        return np.ascontiguousarray(np.stack([q, q], axis=2))

    def wlopair8(wm):  # paired fp8 lo-residuals: [C//(2P), P, 2, ncols]
        hi = wm.astype(NP8).astype(np.float32)
        lo = (wm - hi).astype(NP8)
        pr = lo.reshape(wm.shape[0] // (2 * P), 2, P, wm.shape[1])
        return np.ascontiguousarray(pr.transpose(0, 2, 1, 3))

    rep = {
        "w1q8": wdup8(w1q_m),
        "w1x8": wdup8(w1x_m),
        "w2h8": wdup8(w2h_m),
        "w2qd": wdup8(w2q_m),
        "w2ql": wlopair8(w2q_m),
        "wsqd": wdup8(wsq_m),
        "wsql": wlopair8(wsq_m),
        "wsxd": wdup8(wsx_m),
        "wsxl": wlopair8(wsx_m),
        "b1v": np.ascontiguousarray((SH * b1).reshape(MT, P).T),
        "s2c2v": np.ascontiguousarray((s2 * c2).reshape(OT, P).T),
        "t2v": np.ascontiguousarray(t2.reshape(OT, P).T),
    }

    def act_wide8(aT):
        """(C, BS_core) -> [NB, C//(2P), P, 2(chunk), 2(hi/lo), BLK] fp8."""
        C = aT.shape[0]
        hi, lo = _split8(aT)
        st = np.stack([hi.reshape(C // P, P, NB, BLK),
                       lo.reshape(C // P, P, NB, BLK)], axis=3)
        # [c, p, b, s, col] -> pair: [t, j, p, b, s, col] -> [b, t, p, j, s, col]
        pr = st.reshape(C // (2 * P), 2, P, NB, 2, BLK)
        return np.ascontiguousarray(pr.transpose(3, 0, 2, 1, 4, 5))

    in_maps = []
    for i in range(NCORES):
        sl = slice(i * BS, (i + 1) * BS)
        m = {
            "xw": act_wide8(np.ascontiguousarray(x[sl].T)),
            "q1w": act_wide8(np.ascontiguousarray(q1[sl].T)),
            "q2w": act_wide8(np.ascontiguousarray(q2[sl].T)),
        }
        m.update(rep)
        in_maps.append(m)
    return in_maps


def _run(inputs, trace=False, **trace_kw):
    in_maps = _host_prep(inputs)
    nc = _get_nc()
    res = run_bass_kernel_spmd(nc, in_maps, list(range(NCORES)), trace=trace,
                               **trace_kw)
    # outT per core: [OUT, BS] channel-major -> (BS, OUT)
    out = np.concatenate([r["outT"].T for r in res.results], axis=0)
    return np.ascontiguousarray(out)[:, :, None].astype(np.float32), res


def kernel(**inputs) -> np.ndarray:
    out, _ = _run(inputs, trace=False)
    return out
